# revision 2
# baseline (speedup 1.0000x reference)
"""Trainium2 Bass kernel for ComputeNodeAreaFromPinMap (histogram_binning).

area[n] = sum_{i,j in {0,1}} ox_i * oy_j * U[bx0+i, by0+j]   (2x2 bilinear patch)

Strategy (per core, 8 cores data-parallel over nodes):
  - HOST routes each node to SBUF partition P = 8*q + g where q = bx0 mod 16
    (g round-robin for load balance).  Each GpSimd core c handles q in
    {2c, 2c+1}; its 16 partitions hold the U-table row-sharded mod 16 with
    rho offsets {2c, 2c+1, 2c+2} so both rows bx0 and bx0+1 are present.
  - DVE computes bin indices/overlaps; GpSimd ap_gather fetches 4 table
    values per node (rows bx0/bx0+1 x cols by0/by0+1) as 16-partition slabs.
  - PE one-hot matmuls (16 free-axis phases) collapse each slab back to the
    node's own partition: selection needs no per-node masks because the
    routing invariant makes the source partition a function of phase only.
  - DVE combines: area = ox0*(oy0*UA0+oy1*UA1) + ox1*(oy0*UB0+oy1*UB1).
"""

import sys

sys.path.insert(0, "/opt/trn_rl_repo")

import numpy as np

NM = 2_000_000
NBX = 512
NCORES = 8
S = 2048          # free slots per partition per core (padded)
C = 128           # slots per chunk
NCH = S // C      # chunks
NIDX = 16 * C     # gather indices per core per chunk
NELEM = 32 * 512  # table elements per partition (32 rows x 512 cols)

ROFF = [0] * 6 + [1] * 5 + [2] * 5   # rho offset by u = P % 16
SEL0 = [0] * 8 + [6] * 8             # slab partition (within core) holding row bx0, by phase u
SEL1 = [6] * 8 + [11] * 8            # ... holding row bx0+1

_CACHE = {}


def _build_program():
    import concourse.bacc as bacc
    import concourse.tile as tile
    from concourse import mybir
    f32 = mybir.dt.float32
    i16 = mybir.dt.int16
    alu = mybir.AluOpType

    nc = bacc.Bacc("TRN2", debug=False, target_bir_lowering=False, num_devices=NCORES)

    xin = nc.dram_tensor("x_in", [128, S], f32, kind="ExternalInput").ap()
    yin = nc.dram_tensor("y_in", [128, S], f32, kind="ExternalInput").ap()
    win = nc.dram_tensor("w_in", [128, S], f32, kind="ExternalInput").ap()
    hin = nc.dram_tensor("h_in", [128, S], f32, kind="ExternalInput").ap()
    tab = nc.dram_tensor("tab_in", [128, NELEM], f32, kind="ExternalInput").ap()
    qv = nc.dram_tensor("qv_in", [128, 2], f32, kind="ExternalInput").ap()
    wsel = nc.dram_tensor("wsel_in", [128, 32 * 128], f32, kind="ExternalInput").ap()
    aout = nc.dram_tensor("area_out", [128, S], f32, kind="ExternalOutput").ap()

    with tile.TileContext(nc) as tc, \
         tc.tile_pool(name="const", bufs=1) as cpool, \
         tc.tile_pool(name="work", bufs=2) as wpool, \
         tc.tile_pool(name="slab", bufs=2) as spool, \
         tc.tile_pool(name="psum", bufs=1, space="PSUM") as ppool:

        tab_t = cpool.tile([128, NELEM], f32, tag="tab")
        nc.sync.dma_start(out=tab_t, in_=tab)
        wsel_t = cpool.tile([128, 32 * 128], f32, tag="wsel")
        nc.sync.dma_start(out=wsel_t, in_=wsel)
        qv_t = cpool.tile([128, 2], f32, tag="qv")
        nc.sync.dma_start(out=qv_t, in_=qv)

        for ch in range(NCH):
            sl = slice(ch * C, (ch + 1) * C)
            x = wpool.tile([128, C], f32, tag="x")
            y = wpool.tile([128, C], f32, tag="y")
            w = wpool.tile([128, C], f32, tag="w")
            h = wpool.tile([128, C], f32, tag="h")
            nc.sync.dma_start(out=x, in_=xin[:, sl])
            nc.sync.dma_start(out=y, in_=yin[:, sl])
            nc.sync.dma_start(out=w, in_=win[:, sl])
            nc.sync.dma_start(out=h, in_=hin[:, sl])

            v = nc.vector
            i32 = mybir.dt.int32
            # floor(x*0.5) robust to cast rounding mode: r=cast(t); r -= (r>t)
            t0 = wpool.tile([128, C], f32, tag="t0")
            bi = wpool.tile([128, C], i32, tag="bi")
            bf = wpool.tile([128, C], f32, tag="bf")
            gt = wpool.tile([128, C], f32, tag="gt")
            bx0 = wpool.tile([128, C], f32, tag="bx0")
            by0 = wpool.tile([128, C], f32, tag="by0")
            v.tensor_scalar_mul(t0, x, 0.5)
            v.tensor_copy(bi, t0)
            v.tensor_copy(bf, bi)
            v.tensor_tensor(gt, bf, t0, alu.is_gt)
            v.tensor_sub(bx0, bf, gt)
            v.tensor_scalar_mul(t0, y, 0.5)
            v.tensor_copy(bi, t0)
            v.tensor_copy(bf, bi)
            v.tensor_tensor(gt, bf, t0, alu.is_gt)
            v.tensor_sub(by0, bf, gt)
            tx = wpool.tile([128, C], f32, tag="tx")
            ty = wpool.tile([128, C], f32, tag="ty")
            v.tensor_scalar_mul(tx, bx0, 2.0)
            v.tensor_scalar_mul(ty, by0, 2.0)
            xh = wpool.tile([128, C], f32, tag="xh")
            yh = wpool.tile([128, C], f32, tag="yh")
            v.tensor_add(xh, x, w)
            v.tensor_add(yh, y, h)
            # ox1 = max((xh - tx) - 2, 0) ; ox0 = min(xh, tx+2) - x   (bit-exact vs ref)
            ox1 = wpool.tile([128, C], f32, tag="ox1")
            oy1 = wpool.tile([128, C], f32, tag="oy1")
            v.tensor_sub(t0, xh, tx)
            v.tensor_scalar(ox1, t0, -2.0, 0.0, alu.add, alu.max)
            v.tensor_sub(t0, yh, ty)
            v.tensor_scalar(oy1, t0, -2.0, 0.0, alu.add, alu.max)
            ox0 = wpool.tile([128, C], f32, tag="ox0")
            oy0 = wpool.tile([128, C], f32, tag="oy0")
            v.scalar_tensor_tensor(t0, tx, 2.0, xh, alu.add, alu.min)  # min(tx+2, xh)
            v.tensor_sub(ox0, t0, x)
            v.scalar_tensor_tensor(t0, ty, 2.0, yh, alu.add, alu.min)
            v.tensor_sub(oy0, t0, y)
            # idx0 = (bx0 - q)*32 + by0 ; idx1 = (bx0+1 - (q+1)%16)*32 + by0
            # (q = P//8 is a per-partition constant by the routing invariant)
            idx0 = wpool.tile([128, C], f32, tag="idx0")
            idx1 = wpool.tile([128, C], f32, tag="idx1")
            v.tensor_scalar(t0, bx0, qv_t[:, 0:1], 32.0, alu.subtract, alu.mult)
            v.tensor_add(idx0, t0, by0)
            v.tensor_scalar(t0, bx0, 1.0, None, alu.add)
            v.tensor_scalar(t0, t0, qv_t[:, 1:2], 32.0, alu.subtract, alu.mult)
            v.tensor_add(idx1, t0, by0)

            ii = []
            for k, (src, off) in enumerate(
                [(idx0, 0.0), (idx0, 1.0), (idx1, 0.0), (idx1, 1.0)]
            ):
                it = wpool.tile([128, C], i16, tag=f"ii{k}", name=f"ii{k}_{ch}")
                if off:
                    v.tensor_scalar_add(it, src, off)
                else:
                    v.tensor_copy(it, src)
                ii.append(it)

            slabs = []
            for k in range(4):
                sb = spool.tile([128, NIDX], f32, tag=f"slab{k}", name=f"slab{k}_{ch}")
                nc.gpsimd.ap_gather(
                    sb, tab_t, ii[k], channels=128, num_elems=NELEM, d=1, num_idxs=NIDX
                )
                slabs.append(sb)

            # PE phase-selection: collapse 16-partition slabs to node partitions
            ps = [ppool.tile([128, C], f32, tag=f"ps{k}", name=f"ps{k}_{ch}") for k in range(4)]
            for k in range(4):
                for u in range(16):
                    wk = (u if k < 2 else 16 + u) * 128
                    mv = slabs[k].rearrange("p (s u) -> p u s", u=16)[:, u, :]
                    nc.tensor.matmul(
                        ps[k], wsel_t[:, wk:wk + 128], mv,
                        start=(u == 0), stop=(u == 15),
                    )

            # combine:  area = ox0*(oy0*A0 + oy1*A1) + ox1*(oy0*B0 + oy1*B1)
            tA = wpool.tile([128, C], f32, tag="tA")
            tB = wpool.tile([128, C], f32, tag="tB")
            t1 = wpool.tile([128, C], f32, tag="t1")
            v.tensor_mul(tA, oy0, ps[0])
            v.tensor_mul(t1, oy1, ps[1])
            v.tensor_add(tA, tA, t1)
            v.tensor_mul(tB, oy0, ps[2])
            v.tensor_mul(t1, oy1, ps[3])
            v.tensor_add(tB, tB, t1)
            ar = wpool.tile([128, C], f32, tag="ar")
            v.tensor_mul(ar, ox0, tA)
            v.tensor_mul(t1, ox1, tB)
            v.tensor_add(ar, ar, t1)
            nc.sync.dma_start(out=aout[:, sl], in_=ar)

    nc.compile()
    return nc


def _tables(umap: np.ndarray):
    """Per-partition row-sharded table + selection weights."""
    up = np.zeros((NBX, NBX + 2), np.float32)
    up[:, :NBX] = umap
    tab = np.zeros((128, NELEM), np.float32)
    for P in range(128):
        c = P // 16
        rho = (2 * c + ROFF[P % 16]) % 16
        rows = up[rho::16, :512]                      # [32, 512]
        tab[P] = rows.reshape(-1)
    wsel = np.zeros((32, 128, 128), np.float32)
    for u in range(16):
        for c in range(8):
            wsel[u, 16 * c + SEL0[u], 16 * c + u] = 1.0
            wsel[16 + u, 16 * c + SEL1[u], 16 * c + u] = 1.0
    # stationary layout: [128 part, 32*128]: W_k at cols k*128..k*128+127,
    # W[p, f] -> wsel_flat[p, k*128 + f]
    wsel_flat = np.ascontiguousarray(wsel.transpose(1, 0, 2)).reshape(128, 32 * 128)
    return tab, wsel_flat


def kernel(pos, node_size_x, node_size_y, utilization_map):
    pos = np.asarray(pos, np.float32)
    nsx = np.asarray(node_size_x, np.float32)
    nsy = np.asarray(node_size_y, np.float32)
    umap = np.asarray(utilization_map, np.float32)
    num_nodes = nsx.shape[0]
    x = pos[:NM]
    y = pos[num_nodes:num_nodes + NM]

    if "nc" not in _CACHE:
        _CACHE["nc"] = _build_program()
    nc = _CACHE["nc"]
    tab, wsel = _tables(umap)
    qarr = np.zeros((128, 2), np.float32)
    qarr[:, 0] = np.arange(128) // 8
    qarr[:, 1] = (np.arange(128) // 8 + 1) % 16

    per = NM // NCORES
    in_maps = []
    perms = []
    for cidx in range(NCORES):
        slx = slice(cidx * per, (cidx + 1) * per)
        xc, yc, wc, hc = x[slx], y[slx], nsx[slx], nsy[slx]
        bx0 = np.floor(xc * 0.5).astype(np.int64)
        q = (bx0 & 15).astype(np.int64)
        order = np.argsort(q, kind="stable")
        qs = q[order]
        # position within its q-bucket
        bstart = np.searchsorted(qs, np.arange(16))
        j = np.arange(per) - bstart[qs]
        g = j % 8
        s = j // 8
        P = 8 * qs + g
        assert s.max() < S, f"slot overflow: {s.max()} >= {S}"
        xa = np.zeros((128, S), np.float32)
        ya = np.zeros((128, S), np.float32)
        wa = np.zeros((128, S), np.float32)
        ha = np.zeros((128, S), np.float32)
        xa[P, s] = xc[order]
        ya[P, s] = yc[order]
        wa[P, s] = wc[order]
        ha[P, s] = hc[order]
        in_maps.append(
            {"x_in": xa, "y_in": ya, "w_in": wa, "h_in": ha,
             "tab_in": tab, "wsel_in": wsel, "qv_in": qarr}
        )
        perms.append((order, P, s))

    from concourse import bass_utils

    res = bass_utils.run_bass_kernel_spmd(nc, in_maps, core_ids=list(range(NCORES)))
    _CACHE["last_res"] = res
    out = np.empty(NM, np.float32)
    for cidx in range(NCORES):
        order, P, s = perms[cidx]
        area = res.results[cidx]["area_out"]
        out[cidx * per + order] = area[P, s]
    return out

